# revision 37
# baseline (speedup 1.0000x reference)
"""MoE (E=64, K=8, D=512, I=1024, C=1024) on 8 TRN2 NeuronCores.

Strategy (expert-parallel, per sharding hint):
  - Host: gating (scores/softmax/top-k), dispatch bookkeeping (stable sort by
    expert, capacity slots) and packing of the per-core dispatch buffers.
    Tokens are laid out feature-major ([D, tokens]) so the device kernel
    needs no transposes.
  - Device (SPMD, 8 cores, 8 experts/core): grouped SwiGLU expert GEMMs.
    Stage 1 (x@w1, x@w3) runs in fp8-e4m3 DoubleRow perf mode (2x PE
    throughput, half the weight bytes): w1/w3 are pre-scaled by 32 on the
    host to avoid e4m3 subnormals; the descale rides for free on the silu
    activation's scale argument and on a host-side w2/32 (exact bf16
    exponent shift).  Stage 2 (h@w2) and the data-parallel shared expert
    stay bf16 to keep the overall rel-err ~1.9e-2-safe margin below 2e-2.
  - Host: weighted combine of expert outputs back to token order + shared
    expert add.

kernel(**inputs) takes the FULL unsharded inputs and returns the FULL
[B, S, D] float32 output.
"""

import sys

for _p in ("/opt/trn_rl_repo",):
    if _p not in sys.path:
        sys.path.append(_p)

import numpy as np
import ml_dtypes

import concourse.bacc as bacc
import concourse.mybir as mybir
import concourse.tile as tile
from concourse.bass_utils import run_bass_kernel_spmd

E = 64          # experts
K = 8           # top-k
D = 512         # model dim
I = 1024        # expert inner dim
CAP = 1024      # per-expert capacity in the reference
NCORES = 8
EL = E // NCORES  # experts per core (8)

WSCALE = 32.0   # host pre-scale on w1/w3 before e4m3 quantization

BF16 = mybir.dt.bfloat16
F32 = mybir.dt.float32
F8 = mybir.dt.float8e4
F8NP = ml_dtypes.float8_e4m3   # TRN e4m3: max +-240, matches device fp8e4

# set by test harness: when True, kernel() profiles the NEFF and stores
# exec_time_ns in LAST_EXEC_TIME_NS
TRACE = False
LAST_EXEC_TIME_NS = None
LAST_PROFILE = None

_KERNEL_CACHE = {}


def _install_ntff_hook():
    """antenv.axon_hooks shim so trace=True works under axon here."""
    import types

    try:
        from antenv.axon_hooks import get_axon_ntff_profile_hook  # noqa: F401
    except ImportError:
        import antenv

        m = types.ModuleType("antenv.axon_hooks")
        _store = {}
        m.set_axon_ntff_profile_hook = lambda h: _store.__setitem__("h", h)
        m.get_axon_ntff_profile_hook = lambda: _store.get("h")
        sys.modules["antenv.axon_hooks"] = m
        antenv.axon_hooks = m
    from antenv.axon_hooks import (
        get_axon_ntff_profile_hook,
        set_axon_ntff_profile_hook,
    )

    if get_axon_ntff_profile_hook() is None:
        from trn_agent_boot.trn_boot import _ntff_profile_via_ctypes

        set_axon_ntff_profile_hook(
            _ntff_profile_via_ctypes("/opt/axon/libaxon_pjrt.so")
        )
    from concourse import bass_utils

    bass_utils.upload_artifacts = lambda tmpdir: f"local://{tmpdir}"


def _stage2(nc, psumy_pool, y_pool, w2sb, h_tiles, n_tok, stage2_sink):
    """y = h @ w2 (bf16), psy -> ysb on DVE, store via stage2_sink."""
    n_d = D // 128
    n_i = I // 128
    for m2 in range(n_d):
        psy = psumy_pool.tile([128, n_tok], F32, tag="psy")
        for t2 in range(n_i):
            nc.tensor.matmul(
                psy[:],
                w2sb[:, t2 * D + m2 * 128 : t2 * D + (m2 + 1) * 128],
                h_tiles[t2][:],
                start=(t2 == 0),
                stop=(t2 == n_i - 1),
            )
        ysb = y_pool.tile([128, n_tok], BF16, tag="ysb")
        nc.vector.tensor_copy(ysb[:], psy[:])
        stage2_sink(m2, ysb, n_tok)


def _ffn_block_dr(nc, pools, w1v, w3v, w2sb, xv, xcol0, n_tok, stage2_sink):
    """fp8 DoubleRow stage-1 of one expert for n_tok tokens at column xcol0
    of the fp8 token buffer view xv [128, 4, NTOK]; returns the stage-2
    closure.

    w1v/w3v: [128, 4, I] fp8 views (dim1 = d-subtile)
    w2sb:    [128, 8*D] bf16  (free idx = i_tile*D + d); host pre-divided by
             WSCALE so no extra descale is needed after stage 2.
    """
    psum_pool, psumy_pool, h_pool, s_pool, y_pool = pools
    n_i = I // 128   # 8

    h_tiles = []
    for j in range(n_i):
        ps1 = psum_pool.tile([128, n_tok], F32, tag="ps1")
        ps3 = psum_pool.tile([128, n_tok], F32, tag="ps3")
        for u in range(2):  # d-subtile pairs (0,1) and (2,3)
            nc.tensor.matmul(
                ps1[:],
                w1v[:, 2 * u : 2 * u + 2, j * 128 : (j + 1) * 128],
                xv[:, 2 * u : 2 * u + 2, xcol0 : xcol0 + n_tok],
                start=(u == 0),
                stop=(u == 1),
                perf_mode=mybir.MatmulPerfMode.DoubleRow,
            )
        for u in range(2):
            nc.tensor.matmul(
                ps3[:],
                w3v[:, 2 * u : 2 * u + 2, j * 128 : (j + 1) * 128],
                xv[:, 2 * u : 2 * u + 2, xcol0 : xcol0 + n_tok],
                start=(u == 0),
                stop=(u == 1),
                perf_mode=mybir.MatmulPerfMode.DoubleRow,
            )
        sil = s_pool.tile([128, n_tok], F32, tag="sil")
        # ps1 = WSCALE * (x @ w1): descale inside the activation
        nc.scalar.activation(
            sil[:], ps1[:], mybir.ActivationFunctionType.Silu,
            scale=1.0 / WSCALE,
        )
        h_j = h_pool.tile([128, n_tok], BF16, tag=f"h{j}")
        # h = silu(a) * (WSCALE*b); the stray WSCALE is folded into w2
        nc.vector.tensor_mul(h_j[:], sil[:], ps3[:])
        h_tiles.append(h_j)

    def stage2():
        _stage2(nc, psumy_pool, y_pool, w2sb, h_tiles, n_tok, stage2_sink)

    return stage2


def _ffn_block_bf16(nc, pools, w1sb, w3sb, w2sb, x_tiles, xcol0, n_tok,
                    stage2_sink):
    """bf16 stage-1 (shared expert); returns the stage-2 closure."""
    psum_pool, psumy_pool, h_pool, s_pool, y_pool = pools
    n_d = D // 128   # 4
    n_i = I // 128   # 8

    h_tiles = []
    for j in range(n_i):
        ps1 = psum_pool.tile([128, n_tok], F32, tag="ps1")
        ps3 = psum_pool.tile([128, n_tok], F32, tag="ps3")
        for t in range(n_d):
            rhs = x_tiles[t][:, xcol0 : xcol0 + n_tok]
            nc.tensor.matmul(
                ps1[:],
                w1sb[:, t * I + j * 128 : t * I + (j + 1) * 128],
                rhs,
                start=(t == 0),
                stop=(t == n_d - 1),
            )
        for t in range(n_d):
            rhs = x_tiles[t][:, xcol0 : xcol0 + n_tok]
            nc.tensor.matmul(
                ps3[:],
                w3sb[:, t * I + j * 128 : t * I + (j + 1) * 128],
                rhs,
                start=(t == 0),
                stop=(t == n_d - 1),
            )
        sil = s_pool.tile([128, n_tok], F32, tag="sil")
        nc.scalar.activation(sil[:], ps1[:], mybir.ActivationFunctionType.Silu)
        h_j = h_pool.tile([128, n_tok], BF16, tag=f"hs{j}")
        nc.vector.tensor_mul(h_j[:], sil[:], ps3[:])
        h_tiles.append(h_j)

    def stage2():
        _stage2(nc, psumy_pool, y_pool, w2sb, h_tiles, n_tok, stage2_sink)

    return stage2


def _build(caps, TS):
    """Build the SPMD Bass kernel.

    caps: per-slot token capacities (EL entries; slot = local expert index,
          same across cores -- experts are assigned to slots by load rank so
          padding is minimal)
    TS: shared-expert tokens per core
    DRAM params (per core), weights pre-transposed on host to SBUF
    partition-major layout so their DMAs are flat contiguous copies:
      xbuf [D, sum(caps)] fp8e4   dispatched tokens, feature-major
      w1, w3 [EL, 128, 4*I] fp8e4 (host-scaled by WSCALE)
      w2 [EL, 128, 8*D] bf16 (host-divided by WSCALE)
      xs [4, 128, TS] bf16 ; ws1, ws3 [128, 4*I] bf16 ; ws2 [128, 8*D] bf16
    Outputs:
      yexp [D, sum(caps)] bf16 ; ysh [D, TS] bf16
    """
    NTOK = int(sum(caps))
    offs = [0]
    for c in caps:
        offs.append(offs[-1] + int(c))
    nc = bacc.Bacc("TRN2", target_bir_lowering=False)

    # All weight params arrive pre-transposed into SBUF partition-major
    # layout ([128, free]) so every weight DMA is a flat contiguous copy:
    # strided rearrange DMAs cost multi-microsecond descriptor prep on the
    # issuing engine, flat ones ~0.6us.
    xbuf = nc.declare_dram_parameter("xbuf", [D, NTOK], F8, isOutput=False)
    w1 = nc.declare_dram_parameter("w1", [EL, 128, 4 * I], F8, isOutput=False)
    w3 = nc.declare_dram_parameter("w3", [EL, 128, 4 * I], F8, isOutput=False)
    w2 = nc.declare_dram_parameter("w2", [EL, 128, 8 * D], BF16, isOutput=False)
    xs = nc.declare_dram_parameter("xs", [4, 128, TS], BF16, isOutput=False)
    ws1 = nc.declare_dram_parameter("ws1", [128, 4 * I], BF16, isOutput=False)
    ws3 = nc.declare_dram_parameter("ws3", [128, 4 * I], BF16, isOutput=False)
    ws2 = nc.declare_dram_parameter("ws2", [128, 8 * D], BF16, isOutput=False)
    yexp = nc.declare_dram_parameter("yexp", [D, NTOK], BF16, isOutput=True)
    ysh = nc.declare_dram_parameter("ysh", [D, TS], BF16, isOutput=True)

    n_d = D // 128

    with tile.TileContext(nc) as tc:
        with (
            tc.tile_pool(name="xpool", bufs=1) as xpool,
            tc.tile_pool(name="wpool", bufs=6) as wpool,
            tc.tile_pool(name="w2pool", bufs=4) as w2pool,
            tc.tile_pool(name="wspool", bufs=1) as wspool,
            tc.tile_pool(name="hpool", bufs=3) as h_pool,
            tc.tile_pool(name="hspool", bufs=2) as hs_pool,
            tc.tile_pool(name="spool", bufs=4) as s_pool,
            tc.tile_pool(name="ypool", bufs=4) as y_pool,
            tc.tile_pool(name="psum", bufs=3, space="PSUM") as psum_pool,
            tc.tile_pool(name="psumy", bufs=2, space="PSUM") as psumy_pool,
        ):
            # PE warmup: the PE clock ramp (pstate) is driven by activity,
            # and the first ~7us of the kernel are framework preamble +
            # DMA-wait anyway.  A short burst of dummy matmuls on a memset
            # tile starts the ramp timer early.  Results land in a psum
            # buffer that real work later overwrites (start=True resets).
            warm = xpool.tile([128, 1024], F8, tag="warm")
            nc.gpsimd.memset(warm[:], 0)
            wv = warm[:].rearrange("p (t n) -> p t n", t=2)
            for _ in range(14):
                pw = psumy_pool.tile([128, 512], F32, tag="psy")
                nc.tensor.matmul(
                    pw[:], wv[:, :, 0:128], wv[:, :, 0:512],
                    start=True, stop=True,
                    perf_mode=mybir.MatmulPerfMode.DoubleRow,
                )

            # ALL input transfers ride the single sync queue, in exact
            # deadline order: a queue alone gets the full ~340GB/s (the 16
            # DMA engines round-robin across ACTIVE queues, so spreading
            # inputs over several queues lets late-deadline bytes steal
            # bandwidth from early-deadline ones).  gpsimd's queue carries
            # only the output stores.  Tiles are allocated here; the
            # dma_starts are interleaved with compute emission below so the
            # queue order matches the PE's need order.
            xsb = xpool.tile([128, 4 * NTOK], F8, tag="x")
            xv = xsb[:].rearrange("p (t n) -> p t n", n=NTOK)
            xbuf_r = xbuf.rearrange("(t p) n -> p t n", p=128)
            c1, c2 = offs[1], offs[4]
            # expert-0 token columns first: ~0.16MB, arrives ~0.5us after
            # the queue opens
            nc.sync.dma_start(xv[:, :, 0:c1], xbuf_r[:, :, 0:c1])

            ws1sb = wspool.tile([128, 4 * I], BF16, tag="ws1")
            ws3sb = wspool.tile([128, 4 * I], BF16, tag="ws3")
            ws2sb = wspool.tile([128, 8 * D], BF16, tag="ws2")
            xs_tiles = []
            for t in range(n_d):
                xst = xpool.tile([128, TS], BF16, tag=f"xs{t}")
                xs_tiles.append(xst)

            pools = (psum_pool, psumy_pool, h_pool, s_pool, y_pool)
            sh_pools = (psum_pool, psumy_pool, hs_pool, s_pool, y_pool)

            # token-chunking so a matmul never exceeds 512 moving columns
            def chunks(total):
                out = []
                s = 0
                while s < total:
                    out.append((s, min(512, total - s)))
                    s += 512
                return out

            # Emit order: experts 0..7, then the shared block (its inputs
            # are the tail of the deadline-ordered stream).  For each
            # block: stage1(block) then stage2 of the previous block, so
            # the PE always has independent matmul work while silu/mul of
            # the current block completes.
            pending = []  # stage2 closures not yet emitted

            # w2[le] is only needed at stage2, ~half an expert after w1/w3,
            # so its DMA is staggered one expert late: the issue order on
            # sync is w13[0], w13[1], w2[0], w13[2], w2[1], ... keeping the
            # critical stage-1 weights in front of the queue.
            w2_issue = []  # deferred (w2sb, le) DMA issues

            def issue_w2(w2sb, le):
                nc.sync.dma_start(w2sb[:], w2[le])

            def emit_shared():
                # shared expert (bf16); the final chunk is split small so
                # the downstream stage2 -> cast -> store drains fast.
                sh_chunks = [c for s0, sn in chunks(TS) for c in
                             ([(s0, sn)] if sn <= 128 else
                              [(s0, sn - 128), (s0 + sn - 128, 128)])]
                for s0, sn in sh_chunks:
                    # shared stores go on the (idle-by-now) sync queue so
                    # the end-of-kernel store flush drains two queues in
                    # parallel
                    stage2 = _ffn_block_bf16(
                        nc, sh_pools, ws1sb, ws3sb, ws2sb, xs_tiles, s0, sn,
                        lambda m2, ysb, n_tok, s0=s0: nc.sync.dma_start(
                            ysh[m2 * 128 : (m2 + 1) * 128, s0 : s0 + n_tok],
                            ysb[:, :n_tok],
                        ),
                    )
                    while pending:
                        pending.pop(0)()
                    pending.append(stage2)

            for le in range(EL):
                w1sb = wpool.tile([128, 4 * I], F8, tag="w1")
                w3sb = wpool.tile([128, 4 * I], F8, tag="w3")
                if le <= 1:
                    # early experts: load w1/w3 in i-halves so the first
                    # stage-1 j-tiles depend on only a quarter of the bytes
                    # (the PE is ahead of the DMA stream until ~expert 2)
                    H = I // 2
                    for jh in range(2):
                        for wsb, wsrc in ((w1sb, w1), (w3sb, w3)):
                            nc.sync.dma_start(
                                wsb[:].rearrange("p (t i) -> p t i", i=I)[
                                    :, :, jh * H : (jh + 1) * H
                                ],
                                wsrc[le].rearrange("p (t i) -> p t i", i=I)[
                                    :, :, jh * H : (jh + 1) * H
                                ],
                            )
                else:
                    nc.sync.dma_start(w1sb[:], w1[le])
                    nc.sync.dma_start(w3sb[:], w3[le])
                w2sb = w2pool.tile([128, 8 * D], BF16, tag="w2")
                w2_issue.append((w2sb, le))
                # keep stage-1 weights ahead in the queue: w2[le-1] is
                # issued after w13[le].  (A DMA must be EMITTED before the
                # stage2 that reads its tile — emission order defines the
                # dependency graph — so w2 can only be deferred one expert.)
                if le == 1:
                    # token columns for experts 1-3 (needed from ~20us,
                    # before w2[0])
                    nc.sync.dma_start(xv[:, :, c1:c2], xbuf_r[:, :, c1:c2])
                if le >= 1:
                    issue_w2(*w2_issue.pop(0))
                if le == 2:
                    # token columns for experts 4-7 (needed from ~37us)
                    nc.sync.dma_start(xv[:, :, c2:NTOK], xbuf_r[:, :, c2:NTOK])

                w1v = w1sb[:].rearrange("p (t i) -> p t i", i=I)
                w3v = w3sb[:].rearrange("p (t i) -> p t i", i=I)

                if le == 1:
                    # the PE outruns the DMA stream between expert 0 and
                    # expert 1 (~2us); idle gaps stall the clock ramp, so
                    # fill the wait with dummy matmuls (each <=213ns, so
                    # the real stage-1 start slips by at most one of them)
                    for _ in range(8):
                        pw = psumy_pool.tile([128, 512], F32, tag="psy")
                        nc.tensor.matmul(
                            pw[:], wv[:, :, 0:128], wv[:, :, 0:512],
                            start=True, stop=True,
                            perf_mode=mybir.MatmulPerfMode.DoubleRow,
                        )

                st_eng = nc.gpsimd
                for c0, cn in chunks(int(caps[le])):
                    col0 = offs[le] + c0
                    stage2 = _ffn_block_dr(
                        nc, pools, w1v, w3v, w2sb, xv, col0, cn,
                        lambda m2, ysb, n_tok, col0=col0, st=st_eng: st.dma_start(
                            yexp[m2 * 128 : (m2 + 1) * 128, col0 : col0 + n_tok],
                            ysb[:, :n_tok],
                        ),
                    )
                    # emit previous block's stage2 now (keeps PE busy)
                    while pending:
                        pending.pop(0)()
                    pending.append(stage2)
            # tail of the input stream in deadline order: expert 7's
            # stage-2 runs BEFORE the shared block (emitted via the flush
            # below), covering the shared weights' arrival, so w2[7] comes
            # first, then the shared stage-1 inputs, then ws2
            while w2_issue:
                issue_w2(*w2_issue.pop(0))
            for t in range(n_d):
                nc.sync.dma_start(xs_tiles[t][:], xs[t])
            nc.sync.dma_start(ws1sb[:], ws1[:])
            nc.sync.dma_start(ws3sb[:], ws3[:])
            nc.sync.dma_start(ws2sb[:], ws2[:])
            # emit stage2[7] now so the PE has work while ws1/ws3 land
            while pending:
                pending.pop(0)()
            emit_shared()
            while pending:
                pending.pop(0)()

    nc.compile()
    return nc


def _softmax(x):
    m = x.max(axis=-1, keepdims=True)
    e = np.exp(x - m)
    return e / e.sum(axis=-1, keepdims=True)


def kernel(x, gate_w, adaptive_bias, w1, w3, w2, ws1, ws3, ws2):
    global LAST_EXEC_TIME_NS, LAST_PROFILE

    x = np.asarray(x, dtype=np.float32)
    gate_w = np.asarray(gate_w, dtype=np.float32)
    adaptive_bias = np.asarray(adaptive_bias, dtype=np.float32)
    w1 = np.asarray(w1, dtype=np.float32)
    w3 = np.asarray(w3, dtype=np.float32)
    w2 = np.asarray(w2, dtype=np.float32)
    ws1 = np.asarray(ws1, dtype=np.float32)
    ws3 = np.asarray(ws3, dtype=np.float32)
    ws2 = np.asarray(ws2, dtype=np.float32)

    B, S, _ = x.shape
    T = B * S
    xf = x.reshape(T, D)

    # ---- gating (host, fp32, mirrors reference semantics) ----
    scores = xf @ gate_w.T + adaptive_bias
    probs = _softmax(scores)
    # jax.lax.top_k == stable descending sort, lower index wins ties
    topi = np.argsort(-probs, axis=-1, kind="stable")[:, :K].astype(np.int32)
    topw = np.take_along_axis(probs, topi, axis=-1)
    topw = topw / (topw.sum(axis=-1, keepdims=True) + 1e-8)

    flat_e = topi.reshape(-1)
    flat_w = topw.reshape(-1).astype(np.float32)
    flat_t = np.repeat(np.arange(T), K)

    order = np.argsort(flat_e, kind="stable")
    counts = np.bincount(flat_e, minlength=E)
    offsets = np.cumsum(counts) - counts
    slot_sorted = np.arange(T * K) - offsets[flat_e[order]]
    slot = np.empty(T * K, np.int64)
    slot[order] = slot_sorted
    valid = slot < CAP
    eff_counts = np.minimum(counts, CAP)

    # Assign experts to (core, slot) by load rank: slot s holds the experts
    # ranked [s*NCORES, (s+1)*NCORES), one per core, so every core has the
    # same per-slot capacity with minimal padding.
    perm = np.argsort(-eff_counts, kind="stable")        # expert ids by load desc
    rank = np.empty(E, np.int64)
    rank[perm] = np.arange(E)
    core_of = rank % NCORES
    slot_of = rank // NCORES
    caps = tuple(
        max(4, (int(eff_counts[perm[s * NCORES]]) + 3) // 4 * 4)
        for s in range(EL)
    )
    offs = np.concatenate([[0], np.cumsum(caps)])
    assert T % NCORES == 0
    TS = T // NCORES

    key = (caps, TS)
    if key not in _KERNEL_CACHE:
        _KERNEL_CACHE[key] = _build(caps, TS)
    nc = _KERNEL_CACHE[key]

    # ---- pack per-core inputs (weights pre-transposed to SBUF layout) ----
    def pack_w(a, dt):
        # [E, K128*nt, F] -> [E, 128, nt*F] partition-major
        E_, KD, F_ = a.shape
        nt = KD // 128
        return np.ascontiguousarray(
            a.reshape(E_, nt, 128, F_).transpose(0, 2, 1, 3).reshape(E_, 128, nt * F_)
        ).astype(dt, copy=False)

    def pack_ws(a, dt):
        KD, F_ = a.shape
        nt = KD // 128
        return np.ascontiguousarray(
            a.reshape(nt, 128, F_).transpose(1, 0, 2).reshape(128, nt * F_)
        ).astype(dt, copy=False)

    xb8 = np.clip(xf, -240, 240).astype(F8NP)
    w1_8 = pack_w(np.clip(w1 * WSCALE, -240, 240).astype(F8NP), F8NP)
    w3_8 = pack_w(np.clip(w3 * WSCALE, -240, 240).astype(F8NP), F8NP)
    w2_16 = pack_w((w2 * (1.0 / WSCALE)).astype(ml_dtypes.bfloat16),
                   ml_dtypes.bfloat16)
    xb16 = xf.astype(ml_dtypes.bfloat16)
    ws1_16 = pack_ws(ws1.astype(ml_dtypes.bfloat16), ml_dtypes.bfloat16)
    ws3_16 = pack_ws(ws3.astype(ml_dtypes.bfloat16), ml_dtypes.bfloat16)
    ws2_16 = pack_ws(ws2.astype(ml_dtypes.bfloat16), ml_dtypes.bfloat16)

    NTOK = int(sum(caps))
    v_idx = np.where(valid)[0]
    v_e = flat_e[v_idx]
    v_t = flat_t[v_idx]
    v_slot = slot[v_idx]
    v_core = core_of[v_e]
    v_col = offs[slot_of[v_e]] + v_slot  # column in that core's dispatch buffer

    in_maps = []
    for c in range(NCORES):
        m = v_core == c
        xbuf_c = np.zeros((NTOK, D), dtype=F8NP)
        xbuf_c[v_col[m]] = xb8[v_t[m]]
        experts_c = perm[np.arange(EL) * NCORES + c]  # slot s -> expert id
        in_maps.append(
            {
                "xbuf": np.ascontiguousarray(xbuf_c.T),  # [D, NTOK]
                "w1": np.ascontiguousarray(w1_8[experts_c]),
                "w3": np.ascontiguousarray(w3_8[experts_c]),
                "w2": np.ascontiguousarray(w2_16[experts_c]),
                "xs": np.ascontiguousarray(
                    xb16[c * TS : (c + 1) * TS].T
                ).reshape(4, 128, TS),
                "ws1": ws1_16,
                "ws3": ws3_16,
                "ws2": ws2_16,
            }
        )

    # ---- run on 8 cores ----
    if TRACE:
        _install_ntff_hook()
    res = run_bass_kernel_spmd(
        nc, in_maps, core_ids=list(range(NCORES)), trace=TRACE
    )
    LAST_EXEC_TIME_NS = res.exec_time_ns
    LAST_PROFILE = res
    # yexp per core: [D, NTOK] bf16 ; ysh: [D, TS] bf16
    yexp = np.stack(
        [res.results[c]["yexp"].astype(np.float32) for c in range(NCORES)]
    )
    ysh = np.stack(
        [res.results[c]["ysh"].astype(np.float32) for c in range(NCORES)]
    )

    # ---- combine on host ----
    pair_y = np.zeros((T * K, D), np.float32)
    pair_y[v_idx] = yexp[v_core, :, v_col]  # gather [n_valid, D]
    w_eff = flat_w * valid.astype(np.float32)
    out = (pair_y * w_eff[:, None]).reshape(T, K, D).sum(axis=1)

    shared = ysh.transpose(0, 2, 1).reshape(T, D)
    out = out + shared
    return out.reshape(B, S, D).astype(np.float32)


# revision 38
# speedup vs baseline: 1.0668x; 1.0668x over previous
"""MoE (E=64, K=8, D=512, I=1024, C=1024) on 8 TRN2 NeuronCores.

Strategy (expert-parallel, per sharding hint):
  - Host: gating (scores/softmax/top-k), dispatch bookkeeping (stable sort by
    expert, capacity slots) and packing of the per-core dispatch buffers.
    Tokens are laid out feature-major ([D, tokens]) so the device kernel
    needs no transposes.
  - Device (SPMD, 8 cores, 8 experts/core): grouped SwiGLU expert GEMMs.
    Stage 1 (x@w1, x@w3) runs in fp8-e4m3 DoubleRow perf mode (2x PE
    throughput, half the weight bytes): w1/w3 are pre-scaled by 32 on the
    host to avoid e4m3 subnormals; the descale rides for free on the silu
    activation's scale argument and on a host-side w2/32 (exact bf16
    exponent shift).  Stage 2 (h@w2) and the data-parallel shared expert
    stay bf16 to keep the overall rel-err ~1.9e-2-safe margin below 2e-2.
  - Host: weighted combine of expert outputs back to token order + shared
    expert add.

kernel(**inputs) takes the FULL unsharded inputs and returns the FULL
[B, S, D] float32 output.
"""

import sys

for _p in ("/opt/trn_rl_repo",):
    if _p not in sys.path:
        sys.path.append(_p)

import numpy as np
import ml_dtypes

import concourse.bacc as bacc
import concourse.mybir as mybir
import concourse.tile as tile
from concourse.bass_utils import run_bass_kernel_spmd

E = 64          # experts
K = 8           # top-k
D = 512         # model dim
I = 1024        # expert inner dim
CAP = 1024      # per-expert capacity in the reference
NCORES = 8
EL = E // NCORES  # experts per core (8)

WSCALE = 32.0   # host pre-scale on w1/w3 before e4m3 quantization

BF16 = mybir.dt.bfloat16
F32 = mybir.dt.float32
F8 = mybir.dt.float8e4
F8NP = ml_dtypes.float8_e4m3   # TRN e4m3: max +-240, matches device fp8e4

# set by test harness: when True, kernel() profiles the NEFF and stores
# exec_time_ns in LAST_EXEC_TIME_NS
TRACE = False
LAST_EXEC_TIME_NS = None
LAST_PROFILE = None

_KERNEL_CACHE = {}


def _install_ntff_hook():
    """antenv.axon_hooks shim so trace=True works under axon here."""
    import types

    try:
        from antenv.axon_hooks import get_axon_ntff_profile_hook  # noqa: F401
    except ImportError:
        import antenv

        m = types.ModuleType("antenv.axon_hooks")
        _store = {}
        m.set_axon_ntff_profile_hook = lambda h: _store.__setitem__("h", h)
        m.get_axon_ntff_profile_hook = lambda: _store.get("h")
        sys.modules["antenv.axon_hooks"] = m
        antenv.axon_hooks = m
    from antenv.axon_hooks import (
        get_axon_ntff_profile_hook,
        set_axon_ntff_profile_hook,
    )

    if get_axon_ntff_profile_hook() is None:
        from trn_agent_boot.trn_boot import _ntff_profile_via_ctypes

        set_axon_ntff_profile_hook(
            _ntff_profile_via_ctypes("/opt/axon/libaxon_pjrt.so")
        )
    from concourse import bass_utils

    bass_utils.upload_artifacts = lambda tmpdir: f"local://{tmpdir}"


def _stage2(nc, psumy_pool, y_pool, w2sb, h_tiles, n_tok, stage2_sink):
    """y = h @ w2 (bf16), psy -> ysb on DVE, store via stage2_sink."""
    n_d = D // 128
    n_i = I // 128
    for m2 in range(n_d):
        psy = psumy_pool.tile([128, n_tok], F32, tag="psy")
        for t2 in range(n_i):
            nc.tensor.matmul(
                psy[:],
                w2sb[:, t2 * D + m2 * 128 : t2 * D + (m2 + 1) * 128],
                h_tiles[t2][:],
                start=(t2 == 0),
                stop=(t2 == n_i - 1),
            )
        ysb = y_pool.tile([128, n_tok], BF16, tag="ysb")
        nc.vector.tensor_copy(ysb[:], psy[:])
        stage2_sink(m2, ysb, n_tok)


def _ffn_block_dr(nc, pools, w1v, w3v, w2sb, xv, xcol0, n_tok, stage2_sink):
    """fp8 DoubleRow stage-1 of one expert for n_tok tokens at column xcol0
    of the fp8 token buffer view xv [128, 4, NTOK]; returns the stage-2
    closure.

    w1v/w3v: [128, 4, I] fp8 views (dim1 = d-subtile)
    w2sb:    [128, 8*D] bf16  (free idx = i_tile*D + d); host pre-divided by
             WSCALE so no extra descale is needed after stage 2.
    """
    psum_pool, psumy_pool, h_pool, s_pool, y_pool = pools
    n_i = I // 128   # 8

    h_tiles = []
    for j in range(n_i):
        ps1 = psum_pool.tile([128, n_tok], F32, tag="ps1")
        ps3 = psum_pool.tile([128, n_tok], F32, tag="ps3")
        for u in range(2):  # d-subtile pairs (0,1) and (2,3)
            nc.tensor.matmul(
                ps1[:],
                w1v[:, 2 * u : 2 * u + 2, j * 128 : (j + 1) * 128],
                xv[:, 2 * u : 2 * u + 2, xcol0 : xcol0 + n_tok],
                start=(u == 0),
                stop=(u == 1),
                perf_mode=mybir.MatmulPerfMode.DoubleRow,
            )
        for u in range(2):
            nc.tensor.matmul(
                ps3[:],
                w3v[:, 2 * u : 2 * u + 2, j * 128 : (j + 1) * 128],
                xv[:, 2 * u : 2 * u + 2, xcol0 : xcol0 + n_tok],
                start=(u == 0),
                stop=(u == 1),
                perf_mode=mybir.MatmulPerfMode.DoubleRow,
            )
        sil = s_pool.tile([128, n_tok], F32, tag="sil")
        # ps1 = WSCALE * (x @ w1): descale inside the activation
        nc.scalar.activation(
            sil[:], ps1[:], mybir.ActivationFunctionType.Silu,
            scale=1.0 / WSCALE,
        )
        h_j = h_pool.tile([128, n_tok], BF16, tag=f"h{j}")
        # h = silu(a) * (WSCALE*b); the stray WSCALE is folded into w2
        nc.vector.tensor_mul(h_j[:], sil[:], ps3[:])
        h_tiles.append(h_j)

    def stage2():
        _stage2(nc, psumy_pool, y_pool, w2sb, h_tiles, n_tok, stage2_sink)

    return stage2


def _ffn_block_bf16(nc, pools, w1sb, w3sb, w2sb, x_tiles, xcol0, n_tok,
                    stage2_sink):
    """bf16 stage-1 (shared expert); returns the stage-2 closure."""
    psum_pool, psumy_pool, h_pool, s_pool, y_pool = pools
    n_d = D // 128   # 4
    n_i = I // 128   # 8

    h_tiles = []
    for j in range(n_i):
        ps1 = psum_pool.tile([128, n_tok], F32, tag="ps1")
        ps3 = psum_pool.tile([128, n_tok], F32, tag="ps3")
        for t in range(n_d):
            rhs = x_tiles[t][:, xcol0 : xcol0 + n_tok]
            nc.tensor.matmul(
                ps1[:],
                w1sb[:, t * I + j * 128 : t * I + (j + 1) * 128],
                rhs,
                start=(t == 0),
                stop=(t == n_d - 1),
            )
        for t in range(n_d):
            rhs = x_tiles[t][:, xcol0 : xcol0 + n_tok]
            nc.tensor.matmul(
                ps3[:],
                w3sb[:, t * I + j * 128 : t * I + (j + 1) * 128],
                rhs,
                start=(t == 0),
                stop=(t == n_d - 1),
            )
        sil = s_pool.tile([128, n_tok], F32, tag="sil")
        nc.scalar.activation(sil[:], ps1[:], mybir.ActivationFunctionType.Silu)
        h_j = h_pool.tile([128, n_tok], BF16, tag=f"hs{j}")
        nc.vector.tensor_mul(h_j[:], sil[:], ps3[:])
        h_tiles.append(h_j)

    def stage2():
        _stage2(nc, psumy_pool, y_pool, w2sb, h_tiles, n_tok, stage2_sink)

    return stage2


def _build(caps, TS):
    """Build the SPMD Bass kernel.

    caps: per-slot token capacities (EL entries; slot = local expert index,
          same across cores -- experts are assigned to slots by load rank so
          padding is minimal)
    TS: shared-expert tokens per core
    DRAM params (per core), weights pre-transposed on host to SBUF
    partition-major layout so their DMAs are flat contiguous copies:
      xbuf [D, sum(caps)] fp8e4   dispatched tokens, feature-major
      w1, w3 [EL, 128, 4*I] fp8e4 (host-scaled by WSCALE)
      w2 [EL, 128, 8*D] bf16 (host-divided by WSCALE)
      xs [4, 128, TS] bf16 ; ws1, ws3 [128, 4*I] bf16 ; ws2 [128, 8*D] bf16
    Outputs:
      yexp [D, sum(caps)] bf16 ; ysh [D, TS] bf16
    """
    NTOK = int(sum(caps))
    offs = [0]
    for c in caps:
        offs.append(offs[-1] + int(c))
    nc = bacc.Bacc("TRN2", target_bir_lowering=False)

    # All weight params arrive pre-transposed into SBUF partition-major
    # layout ([128, free]) so every weight DMA is a flat contiguous copy:
    # strided rearrange DMAs cost multi-microsecond descriptor prep on the
    # issuing engine, flat ones ~0.6us.
    xbuf = nc.declare_dram_parameter("xbuf", [D, NTOK], F8, isOutput=False)
    w1 = nc.declare_dram_parameter("w1", [EL, 128, 4 * I], F8, isOutput=False)
    w3 = nc.declare_dram_parameter("w3", [EL, 128, 4 * I], F8, isOutput=False)
    w2 = nc.declare_dram_parameter("w2", [EL, 128, 8 * D], BF16, isOutput=False)
    xs = nc.declare_dram_parameter("xs", [4, 128, TS], BF16, isOutput=False)
    ws1 = nc.declare_dram_parameter("ws1", [128, 4 * I], BF16, isOutput=False)
    ws3 = nc.declare_dram_parameter("ws3", [128, 4 * I], BF16, isOutput=False)
    ws2 = nc.declare_dram_parameter("ws2", [128, 8 * D], BF16, isOutput=False)
    yexp = nc.declare_dram_parameter("yexp", [D, NTOK], BF16, isOutput=True)
    ysh = nc.declare_dram_parameter("ysh", [D, TS], BF16, isOutput=True)

    n_d = D // 128

    with tile.TileContext(nc) as tc:
        with (
            tc.tile_pool(name="xpool", bufs=1) as xpool,
            tc.tile_pool(name="wpool", bufs=6) as wpool,
            tc.tile_pool(name="w2pool", bufs=4) as w2pool,
            tc.tile_pool(name="wspool", bufs=1) as wspool,
            tc.tile_pool(name="hpool", bufs=3) as h_pool,
            tc.tile_pool(name="hspool", bufs=2) as hs_pool,
            tc.tile_pool(name="spool", bufs=4) as s_pool,
            tc.tile_pool(name="ypool", bufs=4) as y_pool,
            tc.tile_pool(name="psum", bufs=3, space="PSUM") as psum_pool,
            tc.tile_pool(name="psumy", bufs=2, space="PSUM") as psumy_pool,
        ):
            # PE warmup: the PE clock ramp (pstate) is driven by activity,
            # and the first ~7us of the kernel are framework preamble +
            # DMA-wait anyway.  A short burst of dummy matmuls on a memset
            # tile starts the ramp timer early.  Results land in a psum
            # buffer that real work later overwrites (start=True resets).
            warm = xpool.tile([128, 1024], F8, tag="warm")
            nc.gpsimd.memset(warm[:], 0)
            wv = warm[:].rearrange("p (t n) -> p t n", t=2)
            for _ in range(14):
                pw = psumy_pool.tile([128, 512], F32, tag="psy")
                nc.tensor.matmul(
                    pw[:], wv[:, :, 0:128], wv[:, :, 0:512],
                    start=True, stop=True,
                    perf_mode=mybir.MatmulPerfMode.DoubleRow,
                )

            # ALL input transfers ride the single sync queue, in exact
            # deadline order: a queue alone gets the full ~340GB/s (the 16
            # DMA engines round-robin across ACTIVE queues, so spreading
            # inputs over several queues lets late-deadline bytes steal
            # bandwidth from early-deadline ones).  gpsimd's queue carries
            # only the output stores.  Tiles are allocated here; the
            # dma_starts are interleaved with compute emission below so the
            # queue order matches the PE's need order.
            xsb = xpool.tile([128, 4 * NTOK], F8, tag="x")
            xv = xsb[:].rearrange("p (t n) -> p t n", n=NTOK)
            xbuf_r = xbuf.rearrange("(t p) n -> p t n", p=128)
            c1, c2 = offs[1], offs[4]
            # expert-0 token columns first: ~0.16MB, arrives ~0.5us after
            # the queue opens
            nc.sync.dma_start(xv[:, :, 0:c1], xbuf_r[:, :, 0:c1])

            ws1sb = wspool.tile([128, 4 * I], BF16, tag="ws1")
            ws3sb = wspool.tile([128, 4 * I], BF16, tag="ws3")
            ws2sb = wspool.tile([128, 8 * D], BF16, tag="ws2")
            xs_tiles = []
            for t in range(n_d):
                xst = xpool.tile([128, TS], BF16, tag=f"xs{t}")
                xs_tiles.append(xst)

            pools = (psum_pool, psumy_pool, h_pool, s_pool, y_pool)
            sh_pools = (psum_pool, psumy_pool, hs_pool, s_pool, y_pool)

            # token-chunking so a matmul never exceeds 512 moving columns
            def chunks(total):
                out = []
                s = 0
                while s < total:
                    out.append((s, min(512, total - s)))
                    s += 512
                return out

            # Emit order: experts 0..7, then the shared block (its inputs
            # are the tail of the deadline-ordered stream).  For each
            # block: stage1(block) then stage2 of the previous block, so
            # the PE always has independent matmul work while silu/mul of
            # the current block completes.
            pending = []  # stage2 closures not yet emitted

            # w2[le] is only needed at stage2, ~half an expert after w1/w3,
            # so its DMA is staggered one expert late: the issue order on
            # sync is w13[0], w13[1], w2[0], w13[2], w2[1], ... keeping the
            # critical stage-1 weights in front of the queue.
            w2_issue = []  # deferred (w2sb, le) DMA issues

            def issue_w2(w2sb, le):
                nc.sync.dma_start(w2sb[:], w2[le])

            def emit_shared():
                # shared expert (bf16); the final chunk is split small so
                # the downstream stage2 -> cast -> store drains fast.
                sh_chunks = [c for s0, sn in chunks(TS) for c in
                             ([(s0, sn)] if sn <= 128 else
                              [(s0, sn - 128), (s0 + sn - 128, 128)])]
                for s0, sn in sh_chunks:
                    # shared stores go on the (idle-by-now) sync queue so
                    # the end-of-kernel store flush drains two queues in
                    # parallel
                    stage2 = _ffn_block_bf16(
                        nc, sh_pools, ws1sb, ws3sb, ws2sb, xs_tiles, s0, sn,
                        lambda m2, ysb, n_tok, s0=s0: nc.sync.dma_start(
                            ysh[m2 * 128 : (m2 + 1) * 128, s0 : s0 + n_tok],
                            ysb[:, :n_tok],
                        ),
                    )
                    while pending:
                        pending.pop(0)()
                    pending.append(stage2)

            for le in range(EL):
                w1sb = wpool.tile([128, 4 * I], F8, tag="w1")
                w3sb = wpool.tile([128, 4 * I], F8, tag="w3")
                if le <= 1:
                    # early experts: load w1/w3 in i-halves so the first
                    # stage-1 j-tiles depend on only a quarter of the bytes
                    # (the PE is ahead of the DMA stream until ~expert 2)
                    H = I // 2
                    for jh in range(2):
                        for wsb, wsrc in ((w1sb, w1), (w3sb, w3)):
                            nc.sync.dma_start(
                                wsb[:].rearrange("p (t i) -> p t i", i=I)[
                                    :, :, jh * H : (jh + 1) * H
                                ],
                                wsrc[le].rearrange("p (t i) -> p t i", i=I)[
                                    :, :, jh * H : (jh + 1) * H
                                ],
                            )
                else:
                    nc.sync.dma_start(w1sb[:], w1[le])
                    nc.sync.dma_start(w3sb[:], w3[le])
                w2sb = w2pool.tile([128, 8 * D], BF16, tag="w2")
                w2_issue.append((w2sb, le))
                # keep stage-1 weights ahead in the queue: w2[le-1] is
                # issued after w13[le].  (A DMA must be EMITTED before the
                # stage2 that reads its tile — emission order defines the
                # dependency graph — so w2 can only be deferred one expert.)
                if le == 1:
                    # token columns for experts 1-3 (needed from ~20us,
                    # before w2[0])
                    nc.sync.dma_start(xv[:, :, c1:c2], xbuf_r[:, :, c1:c2])
                if le >= 1:
                    issue_w2(*w2_issue.pop(0))
                if le == 2:
                    # token columns for experts 4-7 (needed from ~37us)
                    nc.sync.dma_start(xv[:, :, c2:NTOK], xbuf_r[:, :, c2:NTOK])

                w1v = w1sb[:].rearrange("p (t i) -> p t i", i=I)
                w3v = w3sb[:].rearrange("p (t i) -> p t i", i=I)

                if le == 1:
                    # the PE outruns the DMA stream between expert 0 and
                    # expert 1 (~2us); idle gaps stall the clock ramp, so
                    # fill the wait with dummy matmuls (each <=213ns, so
                    # the real stage-1 start slips by at most one of them)
                    for _ in range(8):
                        pw = psumy_pool.tile([128, 512], F32, tag="psy")
                        nc.tensor.matmul(
                            pw[:], wv[:, :, 0:128], wv[:, :, 0:512],
                            start=True, stop=True,
                            perf_mode=mybir.MatmulPerfMode.DoubleRow,
                        )

                st_eng = nc.gpsimd
                for c0, cn in chunks(int(caps[le])):
                    col0 = offs[le] + c0
                    stage2 = _ffn_block_dr(
                        nc, pools, w1v, w3v, w2sb, xv, col0, cn,
                        lambda m2, ysb, n_tok, col0=col0, st=st_eng: st.dma_start(
                            yexp[m2 * 128 : (m2 + 1) * 128, col0 : col0 + n_tok],
                            ysb[:, :n_tok],
                        ),
                    )
                    # emit previous block's stage2 now (keeps PE busy)
                    while pending:
                        pending.pop(0)()
                    pending.append(stage2)
            # tail of the input stream in deadline order: the shared
            # block's stage-1 inputs (xs, ws1/ws3) come before the last
            # expert's w2 and the shared ws2 (both needed later)
            for t in range(n_d):
                nc.sync.dma_start(xs_tiles[t][:], xs[t])
            nc.sync.dma_start(ws1sb[:], ws1[:])
            nc.sync.dma_start(ws3sb[:], ws3[:])
            while w2_issue:
                issue_w2(*w2_issue.pop(0))
            nc.sync.dma_start(ws2sb[:], ws2[:])
            emit_shared()
            while pending:
                pending.pop(0)()

    nc.compile()
    return nc


def _softmax(x):
    m = x.max(axis=-1, keepdims=True)
    e = np.exp(x - m)
    return e / e.sum(axis=-1, keepdims=True)


def kernel(x, gate_w, adaptive_bias, w1, w3, w2, ws1, ws3, ws2):
    global LAST_EXEC_TIME_NS, LAST_PROFILE

    x = np.asarray(x, dtype=np.float32)
    gate_w = np.asarray(gate_w, dtype=np.float32)
    adaptive_bias = np.asarray(adaptive_bias, dtype=np.float32)
    w1 = np.asarray(w1, dtype=np.float32)
    w3 = np.asarray(w3, dtype=np.float32)
    w2 = np.asarray(w2, dtype=np.float32)
    ws1 = np.asarray(ws1, dtype=np.float32)
    ws3 = np.asarray(ws3, dtype=np.float32)
    ws2 = np.asarray(ws2, dtype=np.float32)

    B, S, _ = x.shape
    T = B * S
    xf = x.reshape(T, D)

    # ---- gating (host, fp32, mirrors reference semantics) ----
    scores = xf @ gate_w.T + adaptive_bias
    probs = _softmax(scores)
    # jax.lax.top_k == stable descending sort, lower index wins ties
    topi = np.argsort(-probs, axis=-1, kind="stable")[:, :K].astype(np.int32)
    topw = np.take_along_axis(probs, topi, axis=-1)
    topw = topw / (topw.sum(axis=-1, keepdims=True) + 1e-8)

    flat_e = topi.reshape(-1)
    flat_w = topw.reshape(-1).astype(np.float32)
    flat_t = np.repeat(np.arange(T), K)

    order = np.argsort(flat_e, kind="stable")
    counts = np.bincount(flat_e, minlength=E)
    offsets = np.cumsum(counts) - counts
    slot_sorted = np.arange(T * K) - offsets[flat_e[order]]
    slot = np.empty(T * K, np.int64)
    slot[order] = slot_sorted
    valid = slot < CAP
    eff_counts = np.minimum(counts, CAP)

    # Assign experts to (core, slot) by load rank: slot s holds the experts
    # ranked [s*NCORES, (s+1)*NCORES), one per core, so every core has the
    # same per-slot capacity with minimal padding.
    perm = np.argsort(-eff_counts, kind="stable")        # expert ids by load desc
    rank = np.empty(E, np.int64)
    rank[perm] = np.arange(E)
    core_of = rank % NCORES
    slot_of = rank // NCORES
    caps = tuple(
        max(4, (int(eff_counts[perm[s * NCORES]]) + 3) // 4 * 4)
        for s in range(EL)
    )
    offs = np.concatenate([[0], np.cumsum(caps)])
    assert T % NCORES == 0
    TS = T // NCORES

    key = (caps, TS)
    if key not in _KERNEL_CACHE:
        _KERNEL_CACHE[key] = _build(caps, TS)
    nc = _KERNEL_CACHE[key]

    # ---- pack per-core inputs (weights pre-transposed to SBUF layout) ----
    def pack_w(a, dt):
        # [E, K128*nt, F] -> [E, 128, nt*F] partition-major
        E_, KD, F_ = a.shape
        nt = KD // 128
        return np.ascontiguousarray(
            a.reshape(E_, nt, 128, F_).transpose(0, 2, 1, 3).reshape(E_, 128, nt * F_)
        ).astype(dt, copy=False)

    def pack_ws(a, dt):
        KD, F_ = a.shape
        nt = KD // 128
        return np.ascontiguousarray(
            a.reshape(nt, 128, F_).transpose(1, 0, 2).reshape(128, nt * F_)
        ).astype(dt, copy=False)

    xb8 = np.clip(xf, -240, 240).astype(F8NP)
    w1_8 = pack_w(np.clip(w1 * WSCALE, -240, 240).astype(F8NP), F8NP)
    w3_8 = pack_w(np.clip(w3 * WSCALE, -240, 240).astype(F8NP), F8NP)
    w2_16 = pack_w((w2 * (1.0 / WSCALE)).astype(ml_dtypes.bfloat16),
                   ml_dtypes.bfloat16)
    xb16 = xf.astype(ml_dtypes.bfloat16)
    ws1_16 = pack_ws(ws1.astype(ml_dtypes.bfloat16), ml_dtypes.bfloat16)
    ws3_16 = pack_ws(ws3.astype(ml_dtypes.bfloat16), ml_dtypes.bfloat16)
    ws2_16 = pack_ws(ws2.astype(ml_dtypes.bfloat16), ml_dtypes.bfloat16)

    NTOK = int(sum(caps))
    v_idx = np.where(valid)[0]
    v_e = flat_e[v_idx]
    v_t = flat_t[v_idx]
    v_slot = slot[v_idx]
    v_core = core_of[v_e]
    v_col = offs[slot_of[v_e]] + v_slot  # column in that core's dispatch buffer

    in_maps = []
    for c in range(NCORES):
        m = v_core == c
        xbuf_c = np.zeros((NTOK, D), dtype=F8NP)
        xbuf_c[v_col[m]] = xb8[v_t[m]]
        experts_c = perm[np.arange(EL) * NCORES + c]  # slot s -> expert id
        in_maps.append(
            {
                "xbuf": np.ascontiguousarray(xbuf_c.T),  # [D, NTOK]
                "w1": np.ascontiguousarray(w1_8[experts_c]),
                "w3": np.ascontiguousarray(w3_8[experts_c]),
                "w2": np.ascontiguousarray(w2_16[experts_c]),
                "xs": np.ascontiguousarray(
                    xb16[c * TS : (c + 1) * TS].T
                ).reshape(4, 128, TS),
                "ws1": ws1_16,
                "ws3": ws3_16,
                "ws2": ws2_16,
            }
        )

    # ---- run on 8 cores ----
    if TRACE:
        _install_ntff_hook()
    res = run_bass_kernel_spmd(
        nc, in_maps, core_ids=list(range(NCORES)), trace=TRACE
    )
    LAST_EXEC_TIME_NS = res.exec_time_ns
    LAST_PROFILE = res
    # yexp per core: [D, NTOK] bf16 ; ysh: [D, TS] bf16
    yexp = np.stack(
        [res.results[c]["yexp"].astype(np.float32) for c in range(NCORES)]
    )
    ysh = np.stack(
        [res.results[c]["ysh"].astype(np.float32) for c in range(NCORES)]
    )

    # ---- combine on host ----
    pair_y = np.zeros((T * K, D), np.float32)
    pair_y[v_idx] = yexp[v_core, :, v_col]  # gather [n_valid, D]
    w_eff = flat_w * valid.astype(np.float32)
    out = (pair_y * w_eff[:, None]).reshape(T, K, D).sum(axis=1)

    shared = ysh.transpose(0, 2, 1).reshape(T, D)
    out = out + shared
    return out.reshape(B, S, D).astype(np.float32)
